# revision 12
# baseline (speedup 1.0000x reference)
"""Plan C: PE mask-broadcast + DVE predicated select (bf16) + SWDGE cast store.

Per core (batch element):
  One-time: xwS f32 window-interleave; exact f32 masks e_j = (xw_j == max)
  as bf16; contiguous bf16 xw_j value tensors.
  Per c_pool: 6 K=1 matmuls broadcast mask rows -> PSUM f32; 4 ACT copies
  cast PSUM->SBUF bf16; DVE: base copy + 3 copy_predicated (bf16 2x mode);
  SWDGE cast-DMA bf16->f32 to HBM.
"""

import sys

sys.path.insert(0, "/opt/trn_rl_repo")

import numpy as np

import concourse.bacc as bacc
import concourse.mybir as mybir
import concourse.tile as tile
from concourse.alu_op_type import AluOpType
from concourse.bass_utils import run_bass_kernel_spmd

F32 = mybir.dt.float32
BF16 = mybir.dt.bfloat16
I32 = mybir.dt.int32

C = 128
HW = 4096
P = 1024
HALF = 512
N_CORES = 8

_CACHE = {}


def _build_program():
    nc = bacc.Bacc("TRN2", target_bir_lowering=False)

    x_d = nc.dram_tensor("x", [C, HW], F32, kind="ExternalInput")
    wsel_d = nc.dram_tensor("wsel", [C, C * C], BF16, kind="ExternalInput")
    out_d = nc.dram_tensor("out", [C, C, P], BF16, kind="ExternalOutput")

    with tile.TileContext(nc) as tc:
        with (
            tc.tile_pool(name="persist", bufs=1) as pp,
            tc.tile_pool(name="tmp", bufs=1) as tp,
            tc.tile_pool(name="ots", bufs=4) as op,
            tc.tile_pool(name="mbs", bufs=3) as mp,
            tc.tile_pool(name="psum", bufs=2, space="PSUM") as psp,
        ):
            X = pp.tile([C, HW], F32)
            nc.sync.dma_start(out=X[:], in_=x_d[:])
            wsel = pp.tile([C, C * C], BF16)
            nc.sync.dma_start(out=wsel[:], in_=wsel_d[:])

            X5 = X.rearrange("c (hp dh wp dw) -> c hp dh wp dw",
                             hp=32, dh=2, wp=32, dw=2)

            # window-interleaved f32 copy (exact source for masks)
            xwS = pp.tile([C, HW], F32)
            xwS5 = xwS.rearrange("c (hp wp dh dw) -> c hp wp dh dw",
                                 hp=32, wp=32, dh=2, dw=2)
            for j in range(4):
                nc.vector.tensor_copy(out=xwS5[:, :, :, j // 2, j % 2],
                                      in_=X5[:, :, j // 2, :, j % 2])
            xwS4 = xwS.rearrange("c (i four) -> c i four", four=4)
            xv = [xwS4[:, :, j] for j in range(4)]

            # contiguous bf16 value tensors
            xwb = []
            for j in range(4):
                t = pp.tile([C, P], BF16, name=f"xwb{j}")
                nc.vector.tensor_copy(out=t[:], in_=xv[j])
                xwb.append(t)

            # exact f32 max -> bf16 equality masks
            t0 = tp.tile([C, P], F32)
            t1 = tp.tile([C, P], F32)
            mx = tp.tile([C, P], F32)
            nc.vector.tensor_tensor(out=t0[:], in0=xv[0], in1=xv[1],
                                    op=AluOpType.max)
            nc.vector.tensor_tensor(out=t1[:], in0=xv[2], in1=xv[3],
                                    op=AluOpType.max)
            nc.vector.tensor_tensor(out=mx[:], in0=t0[:], in1=t1[:],
                                    op=AluOpType.max)
            e = []
            for j in range(3):
                ej = pp.tile([C, P], BF16, name=f"e{j}")
                nc.vector.tensor_tensor(out=ej[:], in0=xv[j], in1=mx[:],
                                        op=AluOpType.is_equal)
                e.append(ej)

            for c in range(C):
                wc = wsel[:, c * C:(c + 1) * C]
                mb0 = mp.tile([C, P], BF16, name="mb0")
                mb1 = mp.tile([C, P], BF16, name="mb1")
                mb2 = mp.tile([C, P], BF16, name="mb2")
                mb = (mb0, mb1, mb2)
                for h in range(2):
                    sl = slice(h * HALF, (h + 1) * HALF)
                    ph = psp.tile([C, 3 * HALF], F32, name="ph")
                    for j in range(3):
                        nc.tensor.matmul(ph[:, j * HALF:(j + 1) * HALF],
                                         wc, e[j][:, sl])
                    for j in range(3):
                        nc.scalar.copy(mb[j][:, sl],
                                       ph[:, j * HALF:(j + 1) * HALF])

                if c % 4 == 0:
                    ot = op.tile([C, 4 * P], BF16, name="ot")
                osl = slice((c % 4) * P, (c % 4 + 1) * P)
                nc.vector.tensor_copy(out=ot[:, osl], in_=xwb[3][:])
                nc.vector.copy_predicated(out=ot[:, osl], mask=mb2.bitcast(mybir.dt.int16)[:],
                                          data=xwb[2][:])
                nc.vector.copy_predicated(out=ot[:, osl], mask=mb1.bitcast(mybir.dt.int16)[:],
                                          data=xwb[1][:])
                nc.vector.copy_predicated(out=ot[:, osl], mask=mb0.bitcast(mybir.dt.int16)[:],
                                          data=xwb[0][:])
                if c % 4 == 3:
                    ov = out_d.rearrange("k v i -> v k i")[:, c - 3:c + 1]
                    otv = ot.rearrange("p (k i) -> p k i", k=4)
                    nc.sync.dma_start(out=ov, in_=otv[:])

    nc.compile()
    return nc


def get_program():
    if "nc" not in _CACHE:
        _CACHE["nc"] = _build_program()
    return _CACHE["nc"]


def make_aux_inputs() -> dict:
    import ml_dtypes
    w = np.zeros((C, C, C), dtype=ml_dtypes.bfloat16)
    for k in range(C):
        w[k, k, :] = 1.0
    return {"wsel": w.reshape(C, C * C)}


def kernel(x: np.ndarray) -> np.ndarray:
    assert x.shape == (N_CORES, C, 64, 64), x.shape
    x = np.ascontiguousarray(np.asarray(x, dtype=np.float32))
    nc = get_program()
    aux = make_aux_inputs()
    in_maps = [{"x": x[b].reshape(C, HW), **aux} for b in range(N_CORES)]
    res = run_bass_kernel_spmd(nc, in_maps, core_ids=list(range(N_CORES)))
    out = np.stack([np.asarray(res.results[b]["out"], dtype=np.float32)
                    for b in range(N_CORES)], axis=0)
    return out
